# revision 14
# baseline (speedup 1.0000x reference)
"""Multi-head attention (S=2048, B=2, D=1024, H=16) on 8 Trainium2 NeuronCores.

Sharding: batch*head parallel. Core c owns heads [2c, 2c+1]: it holds the
128-column slice of Wq/Wk/Wv and the matching 128-row slice of Wo, computes
its heads' attention over all tokens, and produces a partial output
projection. Partials are summed on the host (the all-reduce step).

Device layout: activations live transposed (features on partitions, tokens
on the free axis) the whole way through:
  - projections:  QT/KT/VT[oc, t] = W_slice.T @ xT        (oc = head-slice col)
  - scores (transposed): sT[j, i]  = KT_j.T @ QT           (j = key pos chunk)
  - softmax: exp on ScalarE over [128, 1024] tiles (both heads at once to
    amortize the ACT fixed overhead); the row-sum over j comes from an extra
    all-ones column appended to V in the AV matmul; normalization by 1/sum is
    a fast-approx reciprocal + GpSimd partition-broadcast + vector multiply.
  - AV:  ctxT[dk, i] (+ sum row) = [V | 1].T @ exp(sT)     (V in natural [j, dk])
  - output: outT[e, t] = Wo_slice.T @ ctxT
V's natural [token, feature] layout is built from VT with 128x128 PE
transposes.

All matmuls run in float16 operands (fp32 PSUM accumulation). The PE HAM
clock gate doubles matmul rate when the engine stays busy, so the pipeline
is built to avoid tensor idle: QK(j+1) issues before AV(j) (exp off the
critical path), the two heads' K=64 QK matmuls pair into disjoint PE row
groups and run concurrently, projections run dc-outer so the first matmul
only waits for one input tile, and batch 1's input DMAs issue before batch
0's output projection.
"""

import math

import numpy as np

SEQ, BATCH, D_MODEL, HEADS = 2048, 2, 1024, 16
D_K = D_MODEL // HEADS  # 64
N_CORES = 8
HPC = HEADS // N_CORES  # heads per core: 2
OC = HPC * D_K  # per-core head-slice width: 128

LAST_RESULTS = None  # BassKernelResults of the most recent kernel() call


def build_program(S=SEQ, B=BATCH, D=D_MODEL, H=HEADS, n_cores=N_CORES):
    """Build + compile the per-core bass program (SPMD: same program on all
    cores, per-core weight slices arrive via the input maps)."""
    import concourse.bass as bass  # noqa: F401
    import concourse.mybir as mybir
    import concourse.tile as tile
    from concourse import bacc
    from concourse.masks import make_identity

    dk = D // H
    hpc = H // n_cores
    oc = hpc * dk
    T = B * S
    P = 128
    assert oc == P, "kernel assumes a 128-wide per-core head slice"
    assert hpc == 2
    scale = 1.0 / math.sqrt(dk)

    NDC = D // P  # contraction chunks for the projections
    NJC = S // P  # key-position chunks per (batch, head)
    TGW = 512  # token-group width (projections / output)
    NTG_B = S // TGW  # token groups per batch
    EW = 512  # attention i-group width
    NEG = S // EW
    NEC = D // P  # output-projection column chunks

    f32 = mybir.dt.float32
    f16 = mybir.dt.float16
    EXP = mybir.ActivationFunctionType.Exp
    COPY = mybir.ActivationFunctionType.Copy

    nc = bacc.Bacc("TRN2", target_bir_lowering=False, debug=False)

    qT = nc.dram_tensor("qT", (D, T), f16, kind="ExternalInput")
    kT = nc.dram_tensor("kT", (D, T), f16, kind="ExternalInput")
    vT = nc.dram_tensor("vT", (D, T), f16, kind="ExternalInput")
    # weights arrive host-pre-permuted as [p, dc, oc] so one DMA fills the
    # SBUF layout (the sync engine's per-dma_start issue cost was gating the
    # kernel head at 8 chunked loads per weight).
    Wq_s = nc.dram_tensor("Wq_s", (P, NDC, oc), f16, kind="ExternalInput")
    Wk_s = nc.dram_tensor("Wk_s", (P, NDC, oc), f16, kind="ExternalInput")
    Wv_s = nc.dram_tensor("Wv_s", (P, NDC, oc), f16, kind="ExternalInput")
    bq_s = nc.dram_tensor("bq_s", (oc, 1), f32, kind="ExternalInput")
    bk_s = nc.dram_tensor("bk_s", (oc, 1), f32, kind="ExternalInput")
    bv_s = nc.dram_tensor("bv_s", (oc, 1), f32, kind="ExternalInput")
    Wo_s = nc.dram_tensor("Wo_s", (oc, D), f16, kind="ExternalInput")
    outT = nc.dram_tensor("outT", (D, T), f16, kind="ExternalOutput")

    with tile.TileContext(nc) as tc:
        with (
            tc.tile_pool(name="singles", bufs=1) as singles,
            tc.tile_pool(name="xpool", bufs=24) as xpool,
            # 8 bufs: lets the exp stream run ahead of the vaug-gated AV
            # stream at the kernel head (V tiles arrive last).
            tc.tile_pool(name="expool", bufs=8) as expool,
            tc.tile_pool(name="small", bufs=4) as small,
            tc.tile_pool(name="outp", bufs=4) as outp,
            # PSUM (8 banks): tag "qk" = [128,1024]f32 (2 banks) x 2 bufs,
            # attention-exclusive; tag "ctx" = [*,512]f32 (1 bank) x 2 bufs;
            # tag "fill" = [128,512]f32 (1 bank) x 2 bufs for proj ps /
            # transposes / outproj ps2 so those phases never block attention.
            tc.tile_pool(name="psum", bufs=2, space=bass.MemorySpace.PSUM) as psum,
        ):
            ident = singles.tile([P, P], f16)
            make_identity(nc, ident)

            # HAM warm-up: ~30 back-to-back throwaway matmuls so the PE's
            # activity monitor releases the 4/8 clock gate (~3.4us of
            # sustained work) before the DMA-paced projection phase begins.
            # Target: one qk-ring slot, idle until attention starts.
            warm_ps = psum.tile([P, 2 * EW], f32, tag="qk", name="warm_ps")
            for i in range(40):
                r = (i % 8) * P
                nc.tensor.matmul(
                    warm_ps[:, r : r + P], ident[:], ident[:], start=True, stop=True
                )

            wq_sb = singles.tile([P, NDC, oc], f16)
            wk_sb = singles.tile([P, NDC, oc], f16)
            wv_sb = singles.tile([P, NDC, oc], f16)
            wo_sb = singles.tile([oc, D], f16)
            bq_sb = singles.tile([oc, 1], f32)
            bk_sb = singles.tile([oc, 1], f32)
            bv_sb = singles.tile([oc, 1], f32)

            QT_sb = singles.tile([oc, T], f16)
            KT_sb = singles.tile([oc, T], f16)
            VT_sb = singles.tile([oc, T], f16)
            ctxT_sb = singles.tile([oc, T], f16)
            # [V | ones] stationary operands: per (batch, j-chunk) a
            # [128 tokens, hpc*(dk+1)] block, head h at cols h*(dk+1).
            vaug_sb = singles.tile([P, B * NJC, hpc * (dk + 1)], f16)
            ones_cols = singles.tile([P, B * NJC, 1], f32)

            # ---- weight/bias loads: Q's first so proj can start instantly ----
            for w_sb, w_dram, b_sb, b_dram in (
                (wq_sb, Wq_s, bq_sb, bq_s),
                (wk_sb, Wk_s, bk_sb, bk_s),
                (wv_sb, Wv_s, bv_sb, bv_s),
            ):
                nc.sync.dma_start(w_sb[:, :, :], w_dram[:, :, :])
                nc.sync.dma_start(b_sb, b_dram[:, :])
            nc.sync.dma_start(wo_sb, Wo_s[:, :])

            nc.vector.memset(ones_cols, 1.0)
            # preload the exp activation table set (~2.7us) during the
            # DMA-bound head instead of at the first real exp.
            exp_dummy = singles.tile([P, 1], f16)
            nc.scalar.activation(exp_dummy[:], ones_cols[:, 0, :], EXP)
            for h in range(hpc):
                one_col = h * (dk + 1) + dk
                nc.vector.tensor_copy(vaug_sb[:, :, one_col : one_col + 1], ones_cols[:])

            def issue_x_loads(b):
                """Issue input DMAs for batch b's projections: 3 x 8 tiles."""
                xts = {}
                for name, x_dram in (("q", qT), ("k", kT), ("v", vT)):
                    tiles = []
                    for dc in range(NDC):
                        xt = xpool.tile([P, S], f16, tag="xt", name=f"xt_{name}{b}_{dc}")
                        nc.sync.dma_start(
                            xt, x_dram[dc * P : (dc + 1) * P, b * S : (b + 1) * S]
                        )
                        tiles.append(xt)
                    xts[name] = tiles
                return xts

            def run_proj(b, xts):
                """Q/K/V projections for batch b, dc-outer so the first matmul
                only needs one input tile. Two 1024-wide PSUM tiles per proj
                (4 token-group halves); bias-add drains 1024 at a time."""
                for name, w_sb, b_sb, dstT in (
                    ("q", wq_sb, bq_sb, QT_sb),
                    ("k", wk_sb, bk_sb, KT_sb),
                    ("v", wv_sb, bv_sb, VT_sb),
                ):
                    tiles = xts[name]
                    for half in range(NTG_B // 2):
                        pss = [
                            psum.tile(
                                [oc, TGW], f32, tag="fill",
                                name=f"ps_{name}{b}_{half}_{i}",
                            )
                            for i in range(2)
                        ]
                        for dc in range(NDC):
                            for i in range(2):
                                tg = half * 2 + i
                                nc.tensor.matmul(
                                    pss[i],
                                    w_sb[:, dc, :],
                                    tiles[dc][:, tg * TGW : (tg + 1) * TGW],
                                    start=(dc == 0),
                                    stop=(dc == NDC - 1),
                                )
                        for i in range(2):
                            t0 = b * S + (half * 2 + i) * TGW
                            nc.vector.tensor_scalar_add(
                                dstT[:, t0 : t0 + TGW], pss[i], b_sb[:]
                            )

            def run_vaug(b):
                """V natural layout (+ ones cols) via PE transposes."""
                for j in range(NJC):
                    pst = psum.tile([P, P], f16, tag="fill", name=f"pst{b}_{j}")
                    nc.tensor.transpose(
                        pst, VT_sb[:, b * S + j * P : b * S + (j + 1) * P], ident[:]
                    )
                    for h in range(hpc):
                        nc.vector.tensor_copy(
                            vaug_sb[:, b * NJC + j, h * (dk + 1) : h * (dk + 1) + dk],
                            pst[:, h * dk : (h + 1) * dk],
                        )

            def run_attention(b):
                """Attention for both heads of batch b. Software-pipelined:
                QK(j+1) issues before AV(j) so exp(j) (ScalarE) is complete by
                the time the tensor engine reaches AV(j). The two heads' K=64
                QK matmuls land in disjoint PE row groups (h0 rows 0-63, h1
                rows 64-127) writing halves of one 1024-wide PSUM tile, and
                run concurrently; one 1024-wide exp covers both heads."""
                QTp = [QT_sb[h * dk : (h + 1) * dk, b * S : (b + 1) * S] for h in range(hpc)]
                KTp = [KT_sb[h * dk : (h + 1) * dk, b * S : (b + 1) * S] for h in range(hpc)]
                for eg in range(NEG):
                    i0 = eg * EW
                    ctxs = [
                        psum.tile(
                            [dk + 1, EW], f32, tag="ctx", bufs=2, name=f"ctx{b}_{eg}_{h}"
                        )
                        for h in range(hpc)
                    ]

                    def issue_qk(j):
                        qk2 = psum.tile([P, 2 * EW], f32, tag="qk", name=f"qk{b}_{eg}_{j}")
                        for h in range(hpc):
                            nc.tensor.matmul(
                                qk2[:, h * EW : (h + 1) * EW],
                                KTp[h][:, j * P : (j + 1) * P],
                                QTp[h][:, i0 : i0 + EW],
                                start=True,
                                stop=True,
                            )
                        return qk2

                    def issue_exp_av(j, qk2):
                        ex2 = expool.tile([P, 2 * EW], f16, tag="ex", name="ex2")
                        nc.scalar.activation(ex2[:], qk2[:], EXP, scale=scale)
                        for h in range(hpc):
                            nc.tensor.matmul(
                                ctxs[h],
                                vaug_sb[:, b * NJC + j, h * (dk + 1) : (h + 1) * (dk + 1)],
                                ex2[:, h * EW : (h + 1) * EW],
                                start=(j == 0),
                                stop=(j == NJC - 1),
                            )

                    prev = issue_qk(0)
                    for j in range(1, NJC):
                        cur = issue_qk(j)
                        issue_exp_av(j - 1, prev)
                        prev = cur
                    issue_exp_av(NJC - 1, prev)

                    # normalization: 1/sum via fast-approx reciprocal (PSUM
                    # sum row staged through SBUF), partition-broadcast, mul.
                    for h in range(hpc):
                        srow = small.tile([1, EW], f32, tag="srow")
                        nc.vector.tensor_copy(srow[:], ctxs[h][dk : dk + 1, :])
                        rec = small.tile([1, EW], f32, tag="rec")
                        nc.vector.reciprocal_approx_fast(rec[:], srow[:])
                        bc = small.tile([dk, EW], f32, tag="bc")
                        nc.gpsimd.partition_broadcast(bc[:], rec[:])
                        nc.vector.tensor_mul(
                            ctxT_sb[
                                h * dk : (h + 1) * dk, b * S + i0 : b * S + i0 + EW
                            ],
                            ctxs[h][0:dk, :],
                            bc[:],
                        )

            def run_outproj(b, use_scalar_casts):
                """Output projection, ec-outer for stationary reuse, on the
                fill PSUM ring so it can overlap attention. Casts go to DVE
                only when overlapping attention (ScalarE is the exp
                bottleneck there), alternating DVE/ScalarE on the tail."""
                for tg in range(NTG_B):
                    for ec in range(NEC):
                        t0 = b * S + tg * TGW
                        ps2 = psum.tile([P, TGW], f32, tag="fill", name=f"ps2_{b}_{ec}_{tg}")
                        nc.tensor.matmul(
                            ps2,
                            wo_sb[:, ec * P : (ec + 1) * P],
                            ctxT_sb[:, t0 : t0 + TGW],
                            start=True,
                            stop=True,
                        )
                        ot = outp.tile([P, TGW], f16, tag="ot")
                        # ScalarE is the exp bottleneck while attention is in
                        # flight; only the final token-group (the true tail,
                        # after the last exp) may use it.
                        if use_scalar_casts and tg == NTG_B - 1 and ec % 2 == 0:
                            nc.scalar.activation(ot[:], ps2[:], COPY)
                        else:
                            nc.vector.tensor_copy(ot[:], ps2[:])
                        nc.sync.dma_start(
                            outT[ec * P : (ec + 1) * P, t0 : t0 + TGW], ot[:]
                        )

            xts0 = issue_x_loads(0)
            run_proj(0, xts0)
            run_vaug(0)
            run_attention(0)
            # batch 1 input DMAs issue now (sync-queue order!) so they
            # prefetch during batch 0's attention.
            xts1 = issue_x_loads(1)
            run_proj(1, xts1)
            run_vaug(1)
            run_attention(1)
            # issued after attention(1): lower scheduler priority, so these
            # matmuls fill tensor-idle slots of the scalar-paced attention.
            run_outproj(0, use_scalar_casts=False)
            run_outproj(1, use_scalar_casts=True)

    nc.compile()
    return nc


_NC_CACHE = {}


def _compiled():
    if "nc" not in _NC_CACHE:
        _NC_CACHE["nc"] = build_program()
    return _NC_CACHE["nc"]


def _permute_w(w):
    """[D, oc] -> [p, dc, oc] so the device can fetch it with one DMA."""
    D, oc = w.shape
    return np.ascontiguousarray(w.reshape(D // 128, 128, oc).transpose(1, 0, 2))


def make_in_maps(q, k, v, Wq, bq, Wk, bk, Wv, bv, Wo):
    """Shard inputs for the 8 cores: shared transposed activations plus
    per-core head-slice weight columns / Wo rows."""
    f = np.float32
    h16 = np.float16
    qT = np.ascontiguousarray(q.transpose(2, 1, 0).reshape(D_MODEL, -1)).astype(h16)
    kT = np.ascontiguousarray(k.transpose(2, 1, 0).reshape(D_MODEL, -1)).astype(h16)
    vT = np.ascontiguousarray(v.transpose(2, 1, 0).reshape(D_MODEL, -1)).astype(h16)
    Wq, Wk, Wv, Wo = (np.asarray(w).astype(h16) for w in (Wq, Wk, Wv, Wo))
    in_maps = []
    for c in range(N_CORES):
        sl = slice(c * OC, (c + 1) * OC)
        in_maps.append(
            {
                "qT": qT,
                "kT": kT,
                "vT": vT,
                "Wq_s": _permute_w(Wq[:, sl]),
                "Wk_s": _permute_w(Wk[:, sl]),
                "Wv_s": _permute_w(Wv[:, sl]),
                "bq_s": np.ascontiguousarray(bq[sl].reshape(OC, 1), dtype=f),
                "bk_s": np.ascontiguousarray(bk[sl].reshape(OC, 1), dtype=f),
                "bv_s": np.ascontiguousarray(bv[sl].reshape(OC, 1), dtype=f),
                "Wo_s": np.ascontiguousarray(Wo[sl, :]),
            }
        )
    return in_maps


def _install_ntff_shim():
    """Provide antenv.axon_hooks (absent on some images) so that
    trace=True / BASS_TRACE=1 in run_bass_kernel_spmd works instead of
    crashing with ModuleNotFoundError. Best-effort."""
    import sys
    import types

    try:
        import antenv
    except ImportError:
        return
    try:
        import antenv.axon_hooks  # noqa: F401

        return  # real module exists
    except ImportError:
        pass
    mod = types.ModuleType("antenv.axon_hooks")
    mod._hook = None
    mod.set_axon_ntff_profile_hook = lambda h: setattr(mod, "_hook", h)
    mod.get_axon_ntff_profile_hook = lambda: mod._hook
    sys.modules["antenv.axon_hooks"] = mod
    antenv.axon_hooks = mod
    try:
        from trn_agent_boot.trn_boot import _ntff_profile_via_ctypes

        hook = _ntff_profile_via_ctypes("/opt/axon/libaxon_pjrt.so")
        if hook is not None:
            mod.set_axon_ntff_profile_hook(hook)
    except Exception:  # noqa: BLE001
        pass


def kernel(q, k, v, Wq, bq, Wk, bk, Wv, bv, Wo, bo):
    global LAST_RESULTS
    from concourse.bass_utils import run_bass_kernel_spmd

    _install_ntff_shim()

    nc = _compiled()
    in_maps = make_in_maps(q, k, v, Wq, bq, Wk, bk, Wv, bv, Wo)
    res = run_bass_kernel_spmd(nc, in_maps, core_ids=list(range(N_CORES)))
    LAST_RESULTS = res
    total = res.results[0]["outT"].astype(np.float64)
    for c in range(1, N_CORES):
        total += res.results[c]["outT"]
    out = total.reshape(D_MODEL, BATCH, SEQ).transpose(2, 1, 0) + np.asarray(
        bo, dtype=np.float64
    )
    return np.ascontiguousarray(out, dtype=np.float32)


# revision 15
# speedup vs baseline: 1.0690x; 1.0690x over previous
"""Multi-head attention (S=2048, B=2, D=1024, H=16) on 8 Trainium2 NeuronCores.

Sharding: batch*head parallel. Core c owns heads [2c, 2c+1]: it holds the
128-column slice of Wq/Wk/Wv and the matching 128-row slice of Wo, computes
its heads' attention over all tokens, and produces a partial output
projection. Partials are summed on the host (the all-reduce step).

Device layout: activations live transposed (features on partitions, tokens
on the free axis) the whole way through:
  - projections:  QT/KT/VT[oc, t] = W_slice.T @ xT        (oc = head-slice col)
  - scores (transposed): sT[j, i]  = KT_j.T @ QT           (j = key pos chunk)
  - softmax: exp on ScalarE over [128, 1024] tiles (both heads at once to
    amortize the ACT fixed overhead); the row-sum over j comes from an extra
    all-ones column appended to V in the AV matmul; normalization by 1/sum is
    a fast-approx reciprocal + GpSimd partition-broadcast + vector multiply.
  - AV:  ctxT[dk, i] (+ sum row) = [V | 1].T @ exp(sT)     (V in natural [j, dk])
  - output: outT[e, t] = Wo_slice.T @ ctxT
V's natural [token, feature] layout is built from VT with 128x128 PE
transposes.

All matmuls run in float16 operands (fp32 PSUM accumulation). The PE HAM
clock gate doubles matmul rate when the engine stays busy, so the pipeline
is built to avoid tensor idle: QK(j+1) issues before AV(j) (exp off the
critical path), the two heads' K=64 QK matmuls pair into disjoint PE row
groups and run concurrently, projections run dc-outer so the first matmul
only waits for one input tile, and batch 1's input DMAs issue before batch
0's output projection.
"""

import math

import numpy as np

SEQ, BATCH, D_MODEL, HEADS = 2048, 2, 1024, 16
D_K = D_MODEL // HEADS  # 64
N_CORES = 8
HPC = HEADS // N_CORES  # heads per core: 2
OC = HPC * D_K  # per-core head-slice width: 128

LAST_RESULTS = None  # BassKernelResults of the most recent kernel() call


def build_program(S=SEQ, B=BATCH, D=D_MODEL, H=HEADS, n_cores=N_CORES):
    """Build + compile the per-core bass program (SPMD: same program on all
    cores, per-core weight slices arrive via the input maps)."""
    import concourse.bass as bass  # noqa: F401
    import concourse.mybir as mybir
    import concourse.tile as tile
    from concourse import bacc
    from concourse.masks import make_identity

    dk = D // H
    hpc = H // n_cores
    oc = hpc * dk
    T = B * S
    P = 128
    assert oc == P, "kernel assumes a 128-wide per-core head slice"
    assert hpc == 2
    scale = 1.0 / math.sqrt(dk)

    NDC = D // P  # contraction chunks for the projections
    NJC = S // P  # key-position chunks per (batch, head)
    TGW = 512  # token-group width (projections / output)
    NTG_B = S // TGW  # token groups per batch
    EW = 512  # attention i-group width
    NEG = S // EW
    NEC = D // P  # output-projection column chunks

    f32 = mybir.dt.float32
    f16 = mybir.dt.float16
    EXP = mybir.ActivationFunctionType.Exp
    COPY = mybir.ActivationFunctionType.Copy

    nc = bacc.Bacc("TRN2", target_bir_lowering=False, debug=False)

    qT = nc.dram_tensor("qT", (D, T), f16, kind="ExternalInput")
    kT = nc.dram_tensor("kT", (D, T), f16, kind="ExternalInput")
    vT = nc.dram_tensor("vT", (D, T), f16, kind="ExternalInput")
    # weights arrive host-pre-permuted as [p, dc, oc] so one DMA fills the
    # SBUF layout (the sync engine's per-dma_start issue cost was gating the
    # kernel head at 8 chunked loads per weight).
    Wq_s = nc.dram_tensor("Wq_s", (P, NDC, oc), f16, kind="ExternalInput")
    Wk_s = nc.dram_tensor("Wk_s", (P, NDC, oc), f16, kind="ExternalInput")
    Wv_s = nc.dram_tensor("Wv_s", (P, NDC, oc), f16, kind="ExternalInput")
    bq_s = nc.dram_tensor("bq_s", (oc, 1), f32, kind="ExternalInput")
    bk_s = nc.dram_tensor("bk_s", (oc, 1), f32, kind="ExternalInput")
    bv_s = nc.dram_tensor("bv_s", (oc, 1), f32, kind="ExternalInput")
    Wo_s = nc.dram_tensor("Wo_s", (oc, D), f16, kind="ExternalInput")
    outT = nc.dram_tensor("outT", (D, T), f16, kind="ExternalOutput")

    with tile.TileContext(nc) as tc:
        with (
            tc.tile_pool(name="singles", bufs=1) as singles,
            tc.tile_pool(name="xpool", bufs=24) as xpool,
            tc.tile_pool(name="expool", bufs=4) as expool,
            tc.tile_pool(name="small", bufs=4) as small,
            tc.tile_pool(name="outp", bufs=4) as outp,
            # PSUM (8 banks): tag "qk" = [128,1024]f32 (2 banks) x 2 bufs,
            # attention-exclusive; tag "ctx" = [*,512]f32 (1 bank) x 2 bufs;
            # tag "fill" = [128,512]f32 (1 bank) x 2 bufs for proj ps /
            # transposes / outproj ps2 so those phases never block attention.
            tc.tile_pool(name="psum", bufs=2, space=bass.MemorySpace.PSUM) as psum,
        ):
            ident = singles.tile([P, P], f16)
            make_identity(nc, ident)

            # HAM warm-up: ~30 back-to-back throwaway matmuls so the PE's
            # activity monitor releases the 4/8 clock gate (~3.4us of
            # sustained work) before the DMA-paced projection phase begins.
            # Target: one qk-ring slot, idle until attention starts.
            warm_ps = psum.tile([P, 2 * EW], f32, tag="qk", name="warm_ps")
            for i in range(40):
                r = (i % 8) * P
                nc.tensor.matmul(
                    warm_ps[:, r : r + P], ident[:], ident[:], start=True, stop=True
                )

            wq_sb = singles.tile([P, NDC, oc], f16)
            wk_sb = singles.tile([P, NDC, oc], f16)
            wv_sb = singles.tile([P, NDC, oc], f16)
            wo_sb = singles.tile([oc, D], f16)
            bq_sb = singles.tile([oc, 1], f32)
            bk_sb = singles.tile([oc, 1], f32)
            bv_sb = singles.tile([oc, 1], f32)

            QT_sb = singles.tile([oc, T], f16)
            KT_sb = singles.tile([oc, T], f16)
            VT_sb = singles.tile([oc, T], f16)
            ctxT_sb = singles.tile([oc, T], f16)
            # [V | ones] stationary operands: per (batch, j-chunk) a
            # [128 tokens, hpc*(dk+1)] block, head h at cols h*(dk+1).
            vaug_sb = singles.tile([P, B * NJC, hpc * (dk + 1)], f16)
            ones_cols = singles.tile([P, B * NJC, 1], f32)

            # ---- weight/bias loads: Q's first so proj can start instantly ----
            for w_sb, w_dram, b_sb, b_dram in (
                (wq_sb, Wq_s, bq_sb, bq_s),
                (wk_sb, Wk_s, bk_sb, bk_s),
                (wv_sb, Wv_s, bv_sb, bv_s),
            ):
                nc.sync.dma_start(w_sb[:, :, :], w_dram[:, :, :])
                nc.sync.dma_start(b_sb, b_dram[:, :])
            nc.sync.dma_start(wo_sb, Wo_s[:, :])

            nc.vector.memset(ones_cols, 1.0)
            # preload the exp activation table set (~2.7us) during the
            # DMA-bound head instead of at the first real exp.
            exp_dummy = singles.tile([P, 1], f16)
            nc.scalar.activation(exp_dummy[:], ones_cols[:, 0, :], EXP)
            for h in range(hpc):
                one_col = h * (dk + 1) + dk
                nc.vector.tensor_copy(vaug_sb[:, :, one_col : one_col + 1], ones_cols[:])

            def issue_x_loads(b):
                """Issue input DMAs for batch b's projections: 3 x 8 tiles."""
                xts = {}
                for name, x_dram in (("q", qT), ("k", kT), ("v", vT)):
                    tiles = []
                    for dc in range(NDC):
                        xt = xpool.tile([P, S], f16, tag="xt", name=f"xt_{name}{b}_{dc}")
                        nc.sync.dma_start(
                            xt, x_dram[dc * P : (dc + 1) * P, b * S : (b + 1) * S]
                        )
                        tiles.append(xt)
                    xts[name] = tiles
                return xts

            def run_proj(b, xts):
                """Q/K/V projections for batch b, dc-outer so the first matmul
                only needs one input tile. Two 1024-wide PSUM tiles per proj
                (4 token-group halves); bias-add drains 1024 at a time."""
                for name, w_sb, b_sb, dstT in (
                    ("q", wq_sb, bq_sb, QT_sb),
                    ("k", wk_sb, bk_sb, KT_sb),
                    ("v", wv_sb, bv_sb, VT_sb),
                ):
                    tiles = xts[name]
                    for half in range(NTG_B // 2):
                        pss = [
                            psum.tile(
                                [oc, TGW], f32, tag="fill",
                                name=f"ps_{name}{b}_{half}_{i}",
                            )
                            for i in range(2)
                        ]
                        for dc in range(NDC):
                            for i in range(2):
                                tg = half * 2 + i
                                nc.tensor.matmul(
                                    pss[i],
                                    w_sb[:, dc, :],
                                    tiles[dc][:, tg * TGW : (tg + 1) * TGW],
                                    start=(dc == 0),
                                    stop=(dc == NDC - 1),
                                )
                        for i in range(2):
                            t0 = b * S + (half * 2 + i) * TGW
                            nc.vector.tensor_scalar_add(
                                dstT[:, t0 : t0 + TGW], pss[i], b_sb[:]
                            )

            def run_vaug(b):
                """V natural layout (+ ones cols) via PE transposes."""
                for j in range(NJC):
                    pst = psum.tile([P, P], f16, tag="fill", name=f"pst{b}_{j}")
                    nc.tensor.transpose(
                        pst, VT_sb[:, b * S + j * P : b * S + (j + 1) * P], ident[:]
                    )
                    for h in range(hpc):
                        nc.vector.tensor_copy(
                            vaug_sb[:, b * NJC + j, h * (dk + 1) : h * (dk + 1) + dk],
                            pst[:, h * dk : (h + 1) * dk],
                        )

            def run_attention(b):
                """Attention for both heads of batch b. Software-pipelined:
                QK(j+1) issues before AV(j) so exp(j) (ScalarE) is complete by
                the time the tensor engine reaches AV(j). The two heads' K=64
                QK matmuls land in disjoint PE row groups (h0 rows 0-63, h1
                rows 64-127) writing halves of one 1024-wide PSUM tile, and
                run concurrently; one 1024-wide exp covers both heads."""
                QTp = [QT_sb[h * dk : (h + 1) * dk, b * S : (b + 1) * S] for h in range(hpc)]
                KTp = [KT_sb[h * dk : (h + 1) * dk, b * S : (b + 1) * S] for h in range(hpc)]
                for eg in range(NEG):
                    i0 = eg * EW
                    ctxs = [
                        psum.tile(
                            [dk + 1, EW], f32, tag="ctx", bufs=2, name=f"ctx{b}_{eg}_{h}"
                        )
                        for h in range(hpc)
                    ]

                    def issue_qk(j):
                        qk2 = psum.tile([P, 2 * EW], f32, tag="qk", name=f"qk{b}_{eg}_{j}")
                        for h in range(hpc):
                            nc.tensor.matmul(
                                qk2[:, h * EW : (h + 1) * EW],
                                KTp[h][:, j * P : (j + 1) * P],
                                QTp[h][:, i0 : i0 + EW],
                                start=True,
                                stop=True,
                            )
                        return qk2

                    def issue_exp_av(j, qk2):
                        ex2 = expool.tile([P, 2 * EW], f16, tag="ex", name="ex2")
                        nc.scalar.activation(ex2[:], qk2[:], EXP, scale=scale)
                        for h in range(hpc):
                            nc.tensor.matmul(
                                ctxs[h],
                                vaug_sb[:, b * NJC + j, h * (dk + 1) : (h + 1) * (dk + 1)],
                                ex2[:, h * EW : (h + 1) * EW],
                                start=(j == 0),
                                stop=(j == NJC - 1),
                            )

                    prev = issue_qk(0)
                    for j in range(1, NJC):
                        cur = issue_qk(j)
                        issue_exp_av(j - 1, prev)
                        prev = cur
                    issue_exp_av(NJC - 1, prev)

                    # normalization: 1/sum via fast-approx reciprocal (PSUM
                    # sum row staged through SBUF), partition-broadcast, mul.
                    for h in range(hpc):
                        srow = small.tile([1, EW], f32, tag="srow")
                        nc.vector.tensor_copy(srow[:], ctxs[h][dk : dk + 1, :])
                        rec = small.tile([1, EW], f32, tag="rec")
                        nc.vector.reciprocal_approx_fast(rec[:], srow[:])
                        bc = small.tile([dk, EW], f32, tag="bc")
                        nc.gpsimd.partition_broadcast(bc[:], rec[:])
                        nc.vector.tensor_mul(
                            ctxT_sb[
                                h * dk : (h + 1) * dk, b * S + i0 : b * S + i0 + EW
                            ],
                            ctxs[h][0:dk, :],
                            bc[:],
                        )

            def run_outproj(b, use_scalar_casts):
                """Output projection, ec-outer for stationary reuse, on the
                fill PSUM ring so it can overlap attention. Casts go to DVE
                only when overlapping attention (ScalarE is the exp
                bottleneck there), alternating DVE/ScalarE on the tail."""
                for tg in range(NTG_B):
                    for ec in range(NEC):
                        t0 = b * S + tg * TGW
                        ps2 = psum.tile([P, TGW], f32, tag="fill", name=f"ps2_{b}_{ec}_{tg}")
                        nc.tensor.matmul(
                            ps2,
                            wo_sb[:, ec * P : (ec + 1) * P],
                            ctxT_sb[:, t0 : t0 + TGW],
                            start=True,
                            stop=True,
                        )
                        ot = outp.tile([P, TGW], f16, tag="ot")
                        # ScalarE is the exp bottleneck while attention is in
                        # flight; only the final token-group (the true tail,
                        # after the last exp) may use it.
                        if use_scalar_casts and tg == NTG_B - 1 and ec % 2 == 0:
                            nc.scalar.activation(ot[:], ps2[:], COPY)
                        else:
                            nc.vector.tensor_copy(ot[:], ps2[:])
                        nc.sync.dma_start(
                            outT[ec * P : (ec + 1) * P, t0 : t0 + TGW], ot[:]
                        )

            xts0 = issue_x_loads(0)
            run_proj(0, xts0)
            run_vaug(0)
            run_attention(0)
            # batch 1 input DMAs issue now (sync-queue order!) so they
            # prefetch during batch 0's attention.
            xts1 = issue_x_loads(1)
            run_proj(1, xts1)
            run_vaug(1)
            run_attention(1)
            # issued after attention(1): lower scheduler priority, so these
            # matmuls fill tensor-idle slots of the scalar-paced attention.
            run_outproj(0, use_scalar_casts=False)
            run_outproj(1, use_scalar_casts=True)

    nc.compile()
    return nc


_NC_CACHE = {}


def _compiled():
    if "nc" not in _NC_CACHE:
        _NC_CACHE["nc"] = build_program()
    return _NC_CACHE["nc"]


def _permute_w(w):
    """[D, oc] -> [p, dc, oc] so the device can fetch it with one DMA."""
    D, oc = w.shape
    return np.ascontiguousarray(w.reshape(D // 128, 128, oc).transpose(1, 0, 2))


def make_in_maps(q, k, v, Wq, bq, Wk, bk, Wv, bv, Wo):
    """Shard inputs for the 8 cores: shared transposed activations plus
    per-core head-slice weight columns / Wo rows."""
    f = np.float32
    h16 = np.float16
    qT = np.ascontiguousarray(q.transpose(2, 1, 0).reshape(D_MODEL, -1)).astype(h16)
    kT = np.ascontiguousarray(k.transpose(2, 1, 0).reshape(D_MODEL, -1)).astype(h16)
    vT = np.ascontiguousarray(v.transpose(2, 1, 0).reshape(D_MODEL, -1)).astype(h16)
    Wq, Wk, Wv, Wo = (np.asarray(w).astype(h16) for w in (Wq, Wk, Wv, Wo))
    in_maps = []
    for c in range(N_CORES):
        sl = slice(c * OC, (c + 1) * OC)
        in_maps.append(
            {
                "qT": qT,
                "kT": kT,
                "vT": vT,
                "Wq_s": _permute_w(Wq[:, sl]),
                "Wk_s": _permute_w(Wk[:, sl]),
                "Wv_s": _permute_w(Wv[:, sl]),
                "bq_s": np.ascontiguousarray(bq[sl].reshape(OC, 1), dtype=f),
                "bk_s": np.ascontiguousarray(bk[sl].reshape(OC, 1), dtype=f),
                "bv_s": np.ascontiguousarray(bv[sl].reshape(OC, 1), dtype=f),
                "Wo_s": np.ascontiguousarray(Wo[sl, :]),
            }
        )
    return in_maps


def _install_ntff_shim():
    """Provide antenv.axon_hooks (absent on some images) so that
    trace=True / BASS_TRACE=1 in run_bass_kernel_spmd works instead of
    crashing with ModuleNotFoundError. Best-effort."""
    import sys
    import types

    try:
        import antenv
    except ImportError:
        return
    try:
        import antenv.axon_hooks  # noqa: F401

        return  # real module exists
    except ImportError:
        pass
    mod = types.ModuleType("antenv.axon_hooks")
    mod._hook = None
    mod.set_axon_ntff_profile_hook = lambda h: setattr(mod, "_hook", h)
    mod.get_axon_ntff_profile_hook = lambda: mod._hook
    sys.modules["antenv.axon_hooks"] = mod
    antenv.axon_hooks = mod
    try:
        from trn_agent_boot.trn_boot import _ntff_profile_via_ctypes

        hook = _ntff_profile_via_ctypes("/opt/axon/libaxon_pjrt.so")
        if hook is not None:
            mod.set_axon_ntff_profile_hook(hook)
    except Exception:  # noqa: BLE001
        pass


def kernel(q, k, v, Wq, bq, Wk, bk, Wv, bv, Wo, bo):
    global LAST_RESULTS
    from concourse.bass_utils import run_bass_kernel_spmd

    _install_ntff_shim()

    nc = _compiled()
    in_maps = make_in_maps(q, k, v, Wq, bq, Wk, bk, Wv, bv, Wo)
    res = run_bass_kernel_spmd(nc, in_maps, core_ids=list(range(N_CORES)))
    LAST_RESULTS = res
    total = res.results[0]["outT"].astype(np.float64)
    for c in range(1, N_CORES):
        total += res.results[c]["outT"]
    out = total.reshape(D_MODEL, BATCH, SEQ).transpose(2, 1, 0) + np.asarray(
        bo, dtype=np.float64
    )
    return np.ascontiguousarray(out, dtype=np.float32)
